# revision 9
# baseline (speedup 1.0000x reference)
"""BitLinear forward on 8 Trainium2 NeuronCores.

Computation (reference):
    threshold = mean(|W|) * 0.7            (global scalar over full W)
    Wq = sign(W) * (|W| > threshold)       (ternary {-1, 0, 1})
    y = x @ (Wq * scale).T                 (x: [4, 2048, 4096], W: [11008, 4096])

Sharding: column-parallel over out_features. Each core owns a 1376-row slice
of W (zero-padded to 1408 = 11*128), gets the full x, and computes its slice
of the output. The global mean needs a cross-core AllReduce of one scalar.

On-device pipeline per core:
    T: stream W^T tiles, |.|-reduce to a partial sum, AllReduce -> threshold
    Q: re-stream W^T tiles, ternarize to a resident bf16 Wq^T in SBUF (exact)
    M: for each 128-row tile of x: cast x to bf16 (optionally hi+lo split),
       matmul (x tile stationary, Wq^T moving) accumulating over K in PSUM,
       apply scale on PSUM eviction, DMA out.

Matmul dtype: bf16. Wq is exactly representable in bf16 (ternary), scale is
applied in fp32 on the PSUM output, so the only quantization is the x cast.
With SPLIT_LO=True, x is split as x = hi + lo (two bf16 matmuls accumulating
in the same fp32 PSUM) for ~2^-18 relative x error at 2x PE cost.
"""

import numpy as np

import concourse.mybir as mybir
import concourse.tile as tile
from concourse import bacc
from concourse import bass_utils as _bass_utils
from concourse.bass_utils import run_bass_kernel_spmd

# note: walrus --enable-ldw-opt=true rejects bass-emitted standalone
# InstLdweights ("not compatible with LDW optimization"), so the per-matmul
# ~107ns weight load cannot be optimized away at the compiler level.
_ = _bass_utils

N_CORES = 8
O_FULL = 11008
K = 4096
M = 8192
O_SLICE = O_FULL // N_CORES  # 1376
O_PAD = 1408  # 11 * 128
KT = K // 128  # 32
MT = M // 128  # 64
O_CHUNKS = ((0, 512), (512, 512), (1024, 384))
W_COUNT = float(O_FULL) * float(K)
THRESH_FACTOR = 0.7

SPLIT_LO = False  # x = hi + lo bf16 split (2x PE work, ~fp32 accuracy)

_nc_cache = {}


def _build(split_lo: bool):
    nc = bacc.Bacc(None, target_bir_lowering=False)
    f32 = mybir.dt.float32
    bf16 = mybir.dt.bfloat16

    # x pre-tiled on host: xt[mo, ki, ko, mi] = x[mo*128+mi, ko*128+ki]
    xt = nc.dram_tensor("xt", [MT, 128, KT, 128], f32, kind="ExternalInput")
    # W slice transposed: wt[i, o] = W[o_global, i], zero-padded to O_PAD
    wt = nc.dram_tensor("wt", [K, O_PAD], f32, kind="ExternalInput")
    # scale slice replicated to 128 partitions on host
    sc = nc.dram_tensor("sc", [128, O_PAD], f32, kind="ExternalInput")
    y = nc.dram_tensor("y", [M, O_PAD], f32, kind="ExternalOutput")

    wt_t = wt[:].rearrange("(ko ki) o -> ki ko o", ki=128)  # [128, KT, O_PAD]

    with tile.TileContext(nc) as tc:
        with (
            tc.tile_pool(name="const", bufs=1) as const,
            tc.tile_pool(name="wld", bufs=2) as wld,
            tc.tile_pool(name="wldq", bufs=4) as wldq,
            tc.tile_pool(name="qtmp", bufs=2) as qtmp,
            tc.tile_pool(name="wq", bufs=1) as wqp,
            tc.tile_pool(name="xin", bufs=2) as xin,
            tc.tile_pool(name="xbp", bufs=2) as xbp,
            tc.tile_pool(name="yout", bufs=3) as yout,
            tc.tile_pool(name="mm_psum", bufs=2, space="PSUM") as mmps,
            tc.tile_pool(name="sc_psum", bufs=1, space="PSUM") as scps,
            tc.tile_pool(name="dram", bufs=1, space="DRAM") as dram,
        ):
            ones = const.tile([128, 1], f32)
            nc.any.memset(ones[:], 1.0)
            scale_sb = const.tile([128, O_PAD], f32)
            nc.sync.dma_start(scale_sb[:], sc[:])

            # ---- phase T: partial sum of |W| on this core
            acc = const.tile([128, KT], f32)
            for k in range(KT):
                w_k = wld.tile([128, O_PAD], f32, tag="wld")
                nc.sync.dma_start(w_k[:], wt_t[:, k])
                nc.vector.reduce_sum(
                    acc[:, k : k + 1],
                    w_k[:],
                    axis=mybir.AxisListType.X,
                    apply_absolute_value=True,
                )
            red = const.tile([128, 1], f32)
            nc.vector.reduce_sum(red[:], acc[:], axis=mybir.AxisListType.X)
            ps_s = scps.tile([1, 1], f32, tag="s")
            nc.tensor.matmul(ps_s[:], lhsT=ones[:], rhs=red[:], start=True, stop=True)
            part = const.tile([1, 1], f32)
            nc.any.tensor_copy(part[:], ps_s[:])

            # AllGather the 8 per-core partial sums (single collective op),
            # then reduce + broadcast locally.
            cin = dram.tile([1, 1], f32)
            cout = dram.tile([N_CORES, 1], f32, addr_space="Shared")
            nc.sync.dma_start(cin[:], part[:])
            nc.gpsimd.collective_compute(
                "AllGather",
                mybir.AluOpType.bypass,
                ins=[cin.opt()],
                outs=[cout.opt()],
                replica_groups=[list(range(N_CORES))],
            )
            # broadcast the 8 partials to all 128 partitions and sum them:
            # threshold = sum * (1/count) * 0.7
            parts128 = const.tile([128, N_CORES], f32)
            nc.sync.dma_start(
                parts128[:],
                cout[:].rearrange("a b -> b a").to_broadcast((128, N_CORES)),
            )
            tot128 = const.tile([128, 1], f32)
            nc.vector.reduce_sum(tot128[:], parts128[:], axis=mybir.AxisListType.X)
            thr = const.tile([128, 1], f32)
            nc.vector.tensor_scalar(
                thr[:],
                tot128[:],
                float(np.float32(1.0) / np.float32(W_COUNT)),
                THRESH_FACTOR,
                mybir.AluOpType.mult,
                mybir.AluOpType.mult,
            )
            nthr = const.tile([128, 1], f32)
            nc.vector.tensor_scalar_mul(nthr[:], thr[:], -1.0)

            # ---- phase Q: ternarize into resident bf16 Wq^T
            # wq = sign(w - clamp(w, -thr, thr)): exactly 0 for |w| <= thr,
            # else +-1. clamp+sub on DVE, sign on ScalarE (parallel engines).
            # The second W pass prefetches into its own pool so the DMAs run
            # during the collective wait.
            wq_sb = wqp.tile([128, KT, O_PAD], bf16)
            for k in range(KT):
                w_k = wldq.tile([128, O_PAD], f32, tag="wldq")
                nc.sync.dma_start(w_k[:], wt_t[:, k])
                cl = qtmp.tile([128, O_PAD], f32, tag="cl")
                nc.vector.tensor_scalar(
                    cl[:],
                    w_k[:],
                    thr[:],
                    nthr[:],
                    mybir.AluOpType.min,
                    mybir.AluOpType.max,
                )
                df = qtmp.tile([128, O_PAD], bf16, tag="df")
                nc.vector.tensor_tensor(
                    df[:], w_k[:], cl[:], mybir.AluOpType.subtract
                )
                nc.scalar.sign(wq_sb[:, k, :], df[:])

            # ---- phase M: tiled matmul, x stationary / Wq moving
            for mo in range(MT):
                xt_sb = xin.tile([128, KT, 128], f32)
                nc.sync.dma_start(xt_sb[:], xt[mo])
                xb = xbp.tile([128, KT, 128], bf16, tag="hi")
                nc.vector.tensor_copy(xb[:], xt_sb[:])
                if split_lo:
                    xl = xbp.tile([128, KT, 128], bf16, tag="lo")
                    nc.vector.tensor_tensor(
                        xl[:], xt_sb[:], xb[:], mybir.AluOpType.subtract
                    )
                ps = [
                    mmps.tile([128, 512], f32, tag=f"p{ci}", name=f"ps{ci}")
                    for ci in range(len(O_CHUNKS))
                ]
                for k in range(KT):
                    for ci, (o0, w) in enumerate(O_CHUNKS):
                        nc.tensor.matmul(
                            ps[ci][:, :w],
                            lhsT=xb[:, k, :],
                            rhs=wq_sb[:, k, o0 : o0 + w],
                            start=(k == 0),
                            stop=(k == KT - 1 and not split_lo),
                        )
                        if split_lo:
                            nc.tensor.matmul(
                                ps[ci][:, :w],
                                lhsT=xl[:, k, :],
                                rhs=wq_sb[:, k, o0 : o0 + w],
                                start=False,
                                stop=(k == KT - 1),
                            )
                yr = yout.tile([128, O_PAD], f32)
                for ci, (o0, w) in enumerate(O_CHUNKS):
                    nc.vector.tensor_tensor(
                        yr[:, o0 : o0 + w],
                        ps[ci][:, :w],
                        scale_sb[:, o0 : o0 + w],
                        mybir.AluOpType.mult,
                    )
                nc.sync.dma_start(y[mo * 128 : (mo + 1) * 128, :], yr[:])

    nc.compile()
    return nc


def _get_nc(split_lo: bool):
    if split_lo not in _nc_cache:
        _nc_cache[split_lo] = _build(split_lo)
    return _nc_cache[split_lo]


def _prep_inputs(x: np.ndarray, weight: np.ndarray, scale: np.ndarray):
    xf = np.ascontiguousarray(x, dtype=np.float32).reshape(M, K)
    # xt[mo, ki, ko, mi] = x[mo*128+mi, ko*128+ki]
    xt = np.ascontiguousarray(xf.reshape(MT, 128, KT, 128).transpose(0, 3, 2, 1))
    in_maps = []
    for c in range(N_CORES):
        wsl = weight[c * O_SLICE : (c + 1) * O_SLICE].astype(np.float32, copy=False)
        wp = np.zeros((O_PAD, K), dtype=np.float32)
        wp[:O_SLICE] = wsl
        wt = np.ascontiguousarray(wp.T)  # [K, O_PAD]
        ssl = scale[c * O_SLICE : (c + 1) * O_SLICE].astype(np.float32, copy=False)
        sp = np.zeros((O_PAD,), dtype=np.float32)
        sp[:O_SLICE] = ssl.reshape(-1)
        sc = np.ascontiguousarray(np.broadcast_to(sp[None, :], (128, O_PAD)))
        in_maps.append({"xt": xt, "wt": wt, "sc": sc})
    return in_maps


def _run(x, weight, scale, split_lo=None, **run_kwargs):
    if split_lo is None:
        split_lo = SPLIT_LO
    nc = _get_nc(split_lo)
    in_maps = _prep_inputs(x, weight, scale)
    res = run_bass_kernel_spmd(nc, in_maps, core_ids=list(range(N_CORES)), **run_kwargs)
    parts = [res.results[c]["y"][:, :O_SLICE] for c in range(N_CORES)]
    y = np.concatenate(parts, axis=1).reshape(4, 2048, O_FULL).astype(np.float32)
    return y, res


def kernel(x: np.ndarray, weight: np.ndarray, scale: np.ndarray) -> np.ndarray:
    y, _ = _run(x, weight, scale)
    return y


# revision 11
# speedup vs baseline: 1.0279x; 1.0279x over previous
"""BitLinear forward on 8 Trainium2 NeuronCores.

Computation (reference):
    threshold = mean(|W|) * 0.7            (global scalar over full W)
    Wq = sign(W) * (|W| > threshold)       (ternary {-1, 0, 1})
    y = x @ (Wq * scale).T                 (x: [4, 2048, 4096], W: [11008, 4096])

Sharding: column-parallel over out_features. Each core owns a 1376-row slice
of W (zero-padded to 1408 = 11*128), gets the full x, and computes its slice
of the output. The global mean needs a cross-core AllReduce of one scalar.

On-device pipeline per core:
    T: stream W^T tiles, |.|-reduce to a partial sum, AllReduce -> threshold
    Q: re-stream W^T tiles, ternarize to a resident bf16 Wq^T in SBUF (exact)
    M: for each 128-row tile of x: cast x to bf16 (optionally hi+lo split),
       matmul (x tile stationary, Wq^T moving) accumulating over K in PSUM,
       apply scale on PSUM eviction, DMA out.

Matmul dtype: bf16. Wq is exactly representable in bf16 (ternary), scale is
applied in fp32 on the PSUM output, so the only quantization is the x cast.
With SPLIT_LO=True, x is split as x = hi + lo (two bf16 matmuls accumulating
in the same fp32 PSUM) for ~2^-18 relative x error at 2x PE cost.
"""

import numpy as np

import concourse.mybir as mybir
import concourse.tile as tile
from concourse import bacc
from concourse import bass_utils as _bass_utils
from concourse.bass_utils import run_bass_kernel_spmd

# note: walrus --enable-ldw-opt=true rejects bass-emitted standalone
# InstLdweights ("not compatible with LDW optimization"), so the per-matmul
# ~107ns weight load cannot be optimized away at the compiler level.
_ = _bass_utils

N_CORES = 8
O_FULL = 11008
K = 4096
M = 8192
O_SLICE = O_FULL // N_CORES  # 1376
O_PAD = 1408  # 11 * 128
KT = K // 128  # 32
MT = M // 128  # 64
O_CHUNKS = ((0, 512), (512, 512), (1024, 384))
W_COUNT = float(O_FULL) * float(K)
THRESH_FACTOR = 0.7

SPLIT_LO = False  # x = hi + lo bf16 split (2x PE work, ~fp32 accuracy)

_nc_cache = {}


def _build(split_lo: bool):
    nc = bacc.Bacc(None, target_bir_lowering=False)
    f32 = mybir.dt.float32
    bf16 = mybir.dt.bfloat16

    # x pre-tiled on host: xt[mo, ki, ko, mi] = x[mo*128+mi, ko*128+ki]
    xt = nc.dram_tensor("xt", [MT, 128, KT, 128], f32, kind="ExternalInput")
    # W slice transposed: wt[i, o] = W[o_global, i], zero-padded to O_PAD
    wt = nc.dram_tensor("wt", [K, O_PAD], f32, kind="ExternalInput")
    # scale slice replicated to 128 partitions on host
    sc = nc.dram_tensor("sc", [128, O_PAD], f32, kind="ExternalInput")
    y = nc.dram_tensor("y", [M, O_PAD], f32, kind="ExternalOutput")

    wt_t = wt[:].rearrange("(ko ki) o -> ki ko o", ki=128)  # [128, KT, O_PAD]

    with tile.TileContext(nc) as tc:
        with (
            tc.tile_pool(name="const", bufs=1) as const,
            tc.tile_pool(name="wld", bufs=10) as wld,
            tc.tile_pool(name="qtmp", bufs=2) as qtmp,
            tc.tile_pool(name="clp", bufs=1) as clp,
            tc.tile_pool(name="wq", bufs=1) as wqp,
            tc.tile_pool(name="xin", bufs=1) as xin,
            tc.tile_pool(name="xbp", bufs=2) as xbp,
            tc.tile_pool(name="yout", bufs=2) as yout,
            tc.tile_pool(name="mm_psum", bufs=2, space="PSUM") as mmps,
            tc.tile_pool(name="sc_psum", bufs=1, space="PSUM") as scps,
            tc.tile_pool(name="dram", bufs=1, space="DRAM") as dram,
        ):
            ones = const.tile([128, 1], f32)
            nc.any.memset(ones[:], 1.0)
            scale_sb = const.tile([128, O_PAD], f32)
            nc.sync.dma_start(scale_sb[:], sc[:])

            # ---- phase T: partial sum of |W| on this core
            acc = const.tile([128, KT], f32)
            for k in range(KT):
                w_k = wld.tile([128, O_PAD], f32, tag="wld")
                nc.sync.dma_start(w_k[:], wt_t[:, k])
                nc.vector.reduce_sum(
                    acc[:, k : k + 1],
                    w_k[:],
                    axis=mybir.AxisListType.X,
                    apply_absolute_value=True,
                )
            red = const.tile([128, 1], f32)
            nc.vector.reduce_sum(red[:], acc[:], axis=mybir.AxisListType.X)
            ps_s = scps.tile([1, 1], f32, tag="s")
            nc.tensor.matmul(ps_s[:], lhsT=ones[:], rhs=red[:], start=True, stop=True)
            part = const.tile([1, 1], f32)
            nc.any.tensor_copy(part[:], ps_s[:])

            # AllGather the 8 per-core partial sums (single collective op),
            # then reduce + broadcast locally.
            cin = dram.tile([1, 1], f32)
            cout = dram.tile([N_CORES, 1], f32, addr_space="Shared")
            nc.sync.dma_start(cin[:], part[:])
            nc.gpsimd.collective_compute(
                "AllGather",
                mybir.AluOpType.bypass,
                ins=[cin.opt()],
                outs=[cout.opt()],
                replica_groups=[list(range(N_CORES))],
            )
            # broadcast the 8 partials to all 128 partitions and sum them:
            # threshold = sum * (1/count) * 0.7
            parts128 = const.tile([128, N_CORES], f32)
            nc.sync.dma_start(
                parts128[:],
                cout[:].rearrange("a b -> b a").to_broadcast((128, N_CORES)),
            )
            tot128 = const.tile([128, 1], f32)
            nc.vector.reduce_sum(tot128[:], parts128[:], axis=mybir.AxisListType.X)
            thr = const.tile([128, 1], f32)
            nc.vector.tensor_scalar(
                thr[:],
                tot128[:],
                float(np.float32(1.0) / np.float32(W_COUNT)),
                THRESH_FACTOR,
                mybir.AluOpType.mult,
                mybir.AluOpType.mult,
            )
            nthr = const.tile([128, 1], f32)
            nc.vector.tensor_scalar_mul(nthr[:], thr[:], -1.0)

            # ---- phase Q: ternarize into resident bf16 Wq^T
            # wq = sign(w - clamp(w, -thr, thr)): exactly 0 for |w| <= thr,
            # else +-1. clamp+sub on DVE, sign on ScalarE (parallel engines).
            # The second W pass prefetches into its own pool so the DMAs run
            # during the collective wait.
            wq_sb = wqp.tile([128, KT, O_PAD], bf16)
            for k in range(KT):
                w_k = wld.tile([128, O_PAD], f32, tag="wld")
                nc.sync.dma_start(w_k[:], wt_t[:, k])
                cl = clp.tile([128, O_PAD], f32, tag="cl")
                nc.vector.tensor_scalar(
                    cl[:],
                    w_k[:],
                    thr[:],
                    nthr[:],
                    mybir.AluOpType.min,
                    mybir.AluOpType.max,
                )
                df = qtmp.tile([128, O_PAD], bf16, tag="df")
                nc.vector.tensor_tensor(
                    df[:], w_k[:], cl[:], mybir.AluOpType.subtract
                )
                nc.scalar.sign(wq_sb[:, k, :], df[:])

            # ---- phase M: tiled matmul, x stationary / Wq moving
            for mo in range(MT):
                xt_sb = xin.tile([128, KT, 128], f32)
                nc.sync.dma_start(xt_sb[:], xt[mo])
                xb = xbp.tile([128, KT, 128], bf16, tag="hi")
                nc.vector.tensor_copy(xb[:], xt_sb[:])
                if split_lo:
                    xl = xbp.tile([128, KT, 128], bf16, tag="lo")
                    nc.vector.tensor_tensor(
                        xl[:], xt_sb[:], xb[:], mybir.AluOpType.subtract
                    )
                ps = [
                    mmps.tile([128, 512], f32, tag=f"p{ci}", name=f"ps{ci}")
                    for ci in range(len(O_CHUNKS))
                ]
                for k in range(KT):
                    for ci, (o0, w) in enumerate(O_CHUNKS):
                        nc.tensor.matmul(
                            ps[ci][:, :w],
                            lhsT=xb[:, k, :],
                            rhs=wq_sb[:, k, o0 : o0 + w],
                            start=(k == 0),
                            stop=(k == KT - 1 and not split_lo),
                        )
                        if split_lo:
                            nc.tensor.matmul(
                                ps[ci][:, :w],
                                lhsT=xl[:, k, :],
                                rhs=wq_sb[:, k, o0 : o0 + w],
                                start=False,
                                stop=(k == KT - 1),
                            )
                yr = yout.tile([128, O_PAD], f32)
                for ci, (o0, w) in enumerate(O_CHUNKS):
                    nc.vector.tensor_tensor(
                        yr[:, o0 : o0 + w],
                        ps[ci][:, :w],
                        scale_sb[:, o0 : o0 + w],
                        mybir.AluOpType.mult,
                    )
                nc.sync.dma_start(y[mo * 128 : (mo + 1) * 128, :], yr[:])

    nc.compile()
    return nc


def _get_nc(split_lo: bool):
    if split_lo not in _nc_cache:
        _nc_cache[split_lo] = _build(split_lo)
    return _nc_cache[split_lo]


def _prep_inputs(x: np.ndarray, weight: np.ndarray, scale: np.ndarray):
    xf = np.ascontiguousarray(x, dtype=np.float32).reshape(M, K)
    # xt[mo, ki, ko, mi] = x[mo*128+mi, ko*128+ki]
    xt = np.ascontiguousarray(xf.reshape(MT, 128, KT, 128).transpose(0, 3, 2, 1))
    in_maps = []
    for c in range(N_CORES):
        wsl = weight[c * O_SLICE : (c + 1) * O_SLICE].astype(np.float32, copy=False)
        wp = np.zeros((O_PAD, K), dtype=np.float32)
        wp[:O_SLICE] = wsl
        wt = np.ascontiguousarray(wp.T)  # [K, O_PAD]
        ssl = scale[c * O_SLICE : (c + 1) * O_SLICE].astype(np.float32, copy=False)
        sp = np.zeros((O_PAD,), dtype=np.float32)
        sp[:O_SLICE] = ssl.reshape(-1)
        sc = np.ascontiguousarray(np.broadcast_to(sp[None, :], (128, O_PAD)))
        in_maps.append({"xt": xt, "wt": wt, "sc": sc})
    return in_maps


def _run(x, weight, scale, split_lo=None, **run_kwargs):
    if split_lo is None:
        split_lo = SPLIT_LO
    nc = _get_nc(split_lo)
    in_maps = _prep_inputs(x, weight, scale)
    res = run_bass_kernel_spmd(nc, in_maps, core_ids=list(range(N_CORES)), **run_kwargs)
    parts = [res.results[c]["y"][:, :O_SLICE] for c in range(N_CORES)]
    y = np.concatenate(parts, axis=1).reshape(4, 2048, O_FULL).astype(np.float32)
    return y, res


def kernel(x: np.ndarray, weight: np.ndarray, scale: np.ndarray) -> np.ndarray:
    y, _ = _run(x, weight, scale)
    return y


# revision 15
# speedup vs baseline: 1.0294x; 1.0014x over previous
"""BitLinear forward on 8 Trainium2 NeuronCores.

Computation (reference):
    threshold = mean(|W|) * 0.7            (global scalar over full W)
    Wq = sign(W) * (|W| > threshold)       (ternary {-1, 0, 1})
    y = x @ (Wq * scale).T                 (x: [4, 2048, 4096], W: [11008, 4096])

Sharding: column-parallel over out_features. Each core owns a 1376-row slice
of W (zero-padded to 1408 = 11*128), gets the full x, and computes its slice
of the output. The global mean needs a cross-core AllReduce of one scalar.

On-device pipeline per core:
    T: stream W^T tiles, |.|-reduce to a partial sum, AllReduce -> threshold
    Q: re-stream W^T tiles, ternarize to a resident bf16 Wq^T in SBUF (exact)
    M: for each 128-row tile of x: cast x to bf16 (optionally hi+lo split),
       matmul (x tile stationary, Wq^T moving) accumulating over K in PSUM,
       apply scale on PSUM eviction, DMA out.

Matmul dtype: bf16. Wq is exactly representable in bf16 (ternary), scale is
applied in fp32 on the PSUM output, so the only quantization is the x cast.
With SPLIT_LO=True, x is split as x = hi + lo (two bf16 matmuls accumulating
in the same fp32 PSUM) for ~2^-18 relative x error at 2x PE cost.
"""

import numpy as np

import concourse.mybir as mybir
import concourse.tile as tile
from concourse import bacc
from concourse import bass_utils as _bass_utils
from concourse.bass_utils import run_bass_kernel_spmd

# note: walrus --enable-ldw-opt=true rejects bass-emitted standalone
# InstLdweights ("not compatible with LDW optimization"), so the per-matmul
# ~107ns weight load cannot be optimized away at the compiler level.
_ = _bass_utils

N_CORES = 8
O_FULL = 11008
K = 4096
M = 8192
O_SLICE = O_FULL // N_CORES  # 1376
O_PAD = 1408  # 11 * 128
KT = K // 128  # 32
MT = M // 128  # 64
O_CHUNKS = ((0, 512), (512, 512), (1024, 384))
W_COUNT = float(O_FULL) * float(K)
THRESH_FACTOR = 0.7

SPLIT_LO = False  # x = hi + lo bf16 split (2x PE work, ~fp32 accuracy)

_nc_cache = {}


def _build(split_lo: bool):
    nc = bacc.Bacc(None, target_bir_lowering=False)
    f32 = mybir.dt.float32
    bf16 = mybir.dt.bfloat16

    # x pre-tiled on host: xt[mo, ki, ko, mi] = x[mo*128+mi, ko*128+ki]
    xt = nc.dram_tensor("xt", [MT, 128, KT, 128], f32, kind="ExternalInput")
    # W slice transposed: wt[i, o] = W[o_global, i], zero-padded to O_PAD
    wt = nc.dram_tensor("wt", [K, O_PAD], f32, kind="ExternalInput")
    # scale slice replicated to 128 partitions on host
    sc = nc.dram_tensor("sc", [128, O_PAD], f32, kind="ExternalInput")
    y = nc.dram_tensor("y", [M, O_PAD], f32, kind="ExternalOutput")

    wt_t = wt[:].rearrange("(ko ki) o -> ki ko o", ki=128)  # [128, KT, O_PAD]

    with tile.TileContext(nc) as tc:
        with (
            tc.tile_pool(name="const", bufs=1) as const,
            tc.tile_pool(name="wld", bufs=10) as wld,
            tc.tile_pool(name="qtmp", bufs=2) as qtmp,
            tc.tile_pool(name="clp", bufs=1) as clp,
            tc.tile_pool(name="wq", bufs=1) as wqp,
            tc.tile_pool(name="xin", bufs=1) as xin,
            tc.tile_pool(name="xbp", bufs=2) as xbp,
            tc.tile_pool(name="yout", bufs=2) as yout,
            tc.tile_pool(name="mm_psum", bufs=2, space="PSUM") as mmps,
            tc.tile_pool(name="sc_psum", bufs=1, space="PSUM") as scps,
            tc.tile_pool(name="dram", bufs=1, space="DRAM") as dram,
        ):
            ones = const.tile([128, 1], f32)
            nc.any.memset(ones[:], 1.0)
            scale_sb = const.tile([128, O_PAD], f32)
            nc.sync.dma_start(scale_sb[:], sc[:])

            # ---- phase T: partial sum of |W| on this core
            acc = const.tile([128, KT], f32)
            for k in range(KT):
                w_k = wld.tile([128, O_PAD], f32, tag="wld")
                nc.sync.dma_start(w_k[:], wt_t[:, k])
                nc.vector.reduce_sum(
                    acc[:, k : k + 1],
                    w_k[:],
                    axis=mybir.AxisListType.X,
                    apply_absolute_value=True,
                )
            red = const.tile([128, 1], f32)
            nc.vector.reduce_sum(red[:], acc[:], axis=mybir.AxisListType.X)
            ps_s = scps.tile([1, 1], f32, tag="s")
            nc.tensor.matmul(ps_s[:], lhsT=ones[:], rhs=red[:], start=True, stop=True)
            part = const.tile([1, 1], f32)
            nc.any.tensor_copy(part[:], ps_s[:])

            # AllGather the 8 per-core partial sums (single collective op),
            # then reduce + broadcast locally.
            cin = dram.tile([1, 1], f32)
            cout = dram.tile([N_CORES, 1], f32, addr_space="Shared")
            nc.gpsimd.dma_start(cin[:], part[:])
            nc.gpsimd.collective_compute(
                "AllGather",
                mybir.AluOpType.bypass,
                ins=[cin.opt()],
                outs=[cout.opt()],
                replica_groups=[list(range(N_CORES))],
            )
            # broadcast the 8 partials to all 128 partitions and sum them:
            # threshold = sum * (1/count) * 0.7
            parts128 = const.tile([128, N_CORES], f32)
            nc.gpsimd.dma_start(
                parts128[:],
                cout[:].rearrange("a b -> b a").to_broadcast((128, N_CORES)),
            )
            tot128 = const.tile([128, 1], f32)
            nc.vector.reduce_sum(tot128[:], parts128[:], axis=mybir.AxisListType.X)
            thr = const.tile([128, 1], f32)
            nc.vector.tensor_scalar(
                thr[:],
                tot128[:],
                float(np.float32(1.0) / np.float32(W_COUNT)),
                THRESH_FACTOR,
                mybir.AluOpType.mult,
                mybir.AluOpType.mult,
            )
            nthr = const.tile([128, 1], f32)
            nc.vector.tensor_scalar_mul(nthr[:], thr[:], -1.0)

            # ---- phase Q: ternarize into resident bf16 Wq^T
            # wq = sign(w - clamp(w, -thr, thr)): exactly 0 for |w| <= thr,
            # else +-1. clamp+sub on DVE, sign on ScalarE (parallel engines).
            # The second W pass prefetches into its own pool so the DMAs run
            # during the collective wait.
            wq_sb = wqp.tile([128, KT, O_PAD], bf16)
            for k in range(KT):
                w_k = wld.tile([128, O_PAD], f32, tag="wld")
                nc.sync.dma_start(w_k[:], wt_t[:, k])
                cl = clp.tile([128, O_PAD], f32, tag="cl")
                nc.vector.tensor_scalar(
                    cl[:],
                    w_k[:],
                    thr[:],
                    nthr[:],
                    mybir.AluOpType.min,
                    mybir.AluOpType.max,
                )
                df = qtmp.tile([128, O_PAD], bf16, tag="df")
                nc.vector.tensor_tensor(
                    df[:], w_k[:], cl[:], mybir.AluOpType.subtract
                )
                nc.scalar.sign(wq_sb[:, k, :], df[:])

            # ---- phase M: tiled matmul, x stationary / Wq moving
            for mo in range(MT):
                xt_sb = xin.tile([128, KT, 128], f32)
                nc.sync.dma_start(xt_sb[:], xt[mo])
                xb = xbp.tile([128, KT, 128], bf16, tag="hi")
                nc.vector.tensor_copy(xb[:], xt_sb[:])
                if split_lo:
                    xl = xbp.tile([128, KT, 128], bf16, tag="lo")
                    nc.vector.tensor_tensor(
                        xl[:], xt_sb[:], xb[:], mybir.AluOpType.subtract
                    )
                ps = [
                    mmps.tile([128, 512], f32, tag=f"p{ci}", name=f"ps{ci}")
                    for ci in range(len(O_CHUNKS))
                ]
                for k in range(KT):
                    for ci, (o0, w) in enumerate(O_CHUNKS):
                        nc.tensor.matmul(
                            ps[ci][:, :w],
                            lhsT=xb[:, k, :],
                            rhs=wq_sb[:, k, o0 : o0 + w],
                            start=(k == 0),
                            stop=(k == KT - 1 and not split_lo),
                        )
                        if split_lo:
                            nc.tensor.matmul(
                                ps[ci][:, :w],
                                lhsT=xl[:, k, :],
                                rhs=wq_sb[:, k, o0 : o0 + w],
                                start=False,
                                stop=(k == KT - 1),
                            )
                yr = yout.tile([128, O_PAD], f32)
                for ci, (o0, w) in enumerate(O_CHUNKS):
                    nc.vector.tensor_tensor(
                        yr[:, o0 : o0 + w],
                        ps[ci][:, :w],
                        scale_sb[:, o0 : o0 + w],
                        mybir.AluOpType.mult,
                    )
                nc.sync.dma_start(y[mo * 128 : (mo + 1) * 128, :], yr[:])

    nc.compile()
    return nc


def _get_nc(split_lo: bool):
    if split_lo not in _nc_cache:
        _nc_cache[split_lo] = _build(split_lo)
    return _nc_cache[split_lo]


def _prep_inputs(x: np.ndarray, weight: np.ndarray, scale: np.ndarray):
    xf = np.ascontiguousarray(x, dtype=np.float32).reshape(M, K)
    # xt[mo, ki, ko, mi] = x[mo*128+mi, ko*128+ki]
    xt = np.ascontiguousarray(xf.reshape(MT, 128, KT, 128).transpose(0, 3, 2, 1))
    in_maps = []
    for c in range(N_CORES):
        wsl = weight[c * O_SLICE : (c + 1) * O_SLICE].astype(np.float32, copy=False)
        wp = np.zeros((O_PAD, K), dtype=np.float32)
        wp[:O_SLICE] = wsl
        wt = np.ascontiguousarray(wp.T)  # [K, O_PAD]
        ssl = scale[c * O_SLICE : (c + 1) * O_SLICE].astype(np.float32, copy=False)
        sp = np.zeros((O_PAD,), dtype=np.float32)
        sp[:O_SLICE] = ssl.reshape(-1)
        sc = np.ascontiguousarray(np.broadcast_to(sp[None, :], (128, O_PAD)))
        in_maps.append({"xt": xt, "wt": wt, "sc": sc})
    return in_maps


def _run(x, weight, scale, split_lo=None, **run_kwargs):
    if split_lo is None:
        split_lo = SPLIT_LO
    nc = _get_nc(split_lo)
    in_maps = _prep_inputs(x, weight, scale)
    res = run_bass_kernel_spmd(nc, in_maps, core_ids=list(range(N_CORES)), **run_kwargs)
    parts = [res.results[c]["y"][:, :O_SLICE] for c in range(N_CORES)]
    y = np.concatenate(parts, axis=1).reshape(4, 2048, O_FULL).astype(np.float32)
    return y, res


def kernel(x: np.ndarray, weight: np.ndarray, scale: np.ndarray) -> np.ndarray:
    y, _ = _run(x, weight, scale)
    return y


# revision 16
# speedup vs baseline: 1.0304x; 1.0010x over previous
"""BitLinear forward on 8 Trainium2 NeuronCores.

Computation (reference):
    threshold = mean(|W|) * 0.7            (global scalar over full W)
    Wq = sign(W) * (|W| > threshold)       (ternary {-1, 0, 1})
    y = x @ (Wq * scale).T                 (x: [4, 2048, 4096], W: [11008, 4096])

Sharding: column-parallel over out_features. Each core owns a 1376-row slice
of W (zero-padded to 1408 = 11*128), gets the full x, and computes its slice
of the output. The global mean needs a cross-core AllReduce of one scalar.

On-device pipeline per core:
    T: stream W^T tiles, |.|-reduce to a partial sum, AllReduce -> threshold
    Q: re-stream W^T tiles, ternarize to a resident bf16 Wq^T in SBUF (exact)
    M: for each 128-row tile of x: cast x to bf16 (optionally hi+lo split),
       matmul (x tile stationary, Wq^T moving) accumulating over K in PSUM,
       apply scale on PSUM eviction, DMA out.

Matmul dtype: bf16. Wq is exactly representable in bf16 (ternary), scale is
applied in fp32 on the PSUM output, so the only quantization is the x cast.
With SPLIT_LO=True, x is split as x = hi + lo (two bf16 matmuls accumulating
in the same fp32 PSUM) for ~2^-18 relative x error at 2x PE cost.
"""

import numpy as np

import concourse.mybir as mybir
import concourse.tile as tile
from concourse import bacc
from concourse import bass_utils as _bass_utils
from concourse.bass_utils import run_bass_kernel_spmd

# note: walrus --enable-ldw-opt=true rejects bass-emitted standalone
# InstLdweights ("not compatible with LDW optimization"), so the per-matmul
# ~107ns weight load cannot be optimized away at the compiler level.
_ = _bass_utils

N_CORES = 8
O_FULL = 11008
K = 4096
M = 8192
O_SLICE = O_FULL // N_CORES  # 1376
O_PAD = 1408  # 11 * 128
KT = K // 128  # 32
MT = M // 128  # 64
O_CHUNKS = ((0, 512), (512, 512), (1024, 384))
W_COUNT = float(O_FULL) * float(K)
THRESH_FACTOR = 0.7

SPLIT_LO = False  # x = hi + lo bf16 split (2x PE work, ~fp32 accuracy)

_nc_cache = {}


def _build(split_lo: bool):
    nc = bacc.Bacc(None, target_bir_lowering=False)
    f32 = mybir.dt.float32
    bf16 = mybir.dt.bfloat16
    f16 = mybir.dt.float16

    # x pre-tiled on host: xt[mo, ki, ko, mi] = x[mo*128+mi, ko*128+ki]
    xt = nc.dram_tensor("xt", [MT, 128, KT, 128], f32, kind="ExternalInput")
    # W slice transposed: wt[i, o] = W[o_global, i], zero-padded to O_PAD
    wt = nc.dram_tensor("wt", [K, O_PAD], f32, kind="ExternalInput")
    # scale slice replicated to 128 partitions on host
    sc = nc.dram_tensor("sc", [128, O_PAD], f32, kind="ExternalInput")
    y = nc.dram_tensor("y", [M, O_PAD], f32, kind="ExternalOutput")

    wt_t = wt[:].rearrange("(ko ki) o -> ki ko o", ki=128)  # [128, KT, O_PAD]

    with tile.TileContext(nc) as tc:
        with (
            tc.tile_pool(name="const", bufs=1) as const,
            tc.tile_pool(name="wld", bufs=10) as wld,
            tc.tile_pool(name="qtmp", bufs=2) as qtmp,
            tc.tile_pool(name="clp", bufs=1) as clp,
            tc.tile_pool(name="wq", bufs=1) as wqp,
            tc.tile_pool(name="xin", bufs=1) as xin,
            tc.tile_pool(name="xbp", bufs=2) as xbp,
            tc.tile_pool(name="yout", bufs=2) as yout,
            tc.tile_pool(name="mm_psum", bufs=2, space="PSUM") as mmps,
            tc.tile_pool(name="sc_psum", bufs=1, space="PSUM") as scps,
            tc.tile_pool(name="dram", bufs=1, space="DRAM") as dram,
        ):
            ones = const.tile([128, 1], f32)
            nc.any.memset(ones[:], 1.0)
            scale_sb = const.tile([128, O_PAD], f32)
            nc.sync.dma_start(scale_sb[:], sc[:])

            # ---- phase T: partial sum of |W| on this core
            acc = const.tile([128, KT], f32)
            for k in range(KT):
                w_k = wld.tile([128, O_PAD], f32, tag="wld")
                nc.sync.dma_start(w_k[:], wt_t[:, k])
                nc.vector.reduce_sum(
                    acc[:, k : k + 1],
                    w_k[:],
                    axis=mybir.AxisListType.X,
                    apply_absolute_value=True,
                )
            red = const.tile([128, 1], f32)
            nc.vector.reduce_sum(red[:], acc[:], axis=mybir.AxisListType.X)
            ps_s = scps.tile([1, 1], f32, tag="s")
            nc.tensor.matmul(ps_s[:], lhsT=ones[:], rhs=red[:], start=True, stop=True)
            part = const.tile([1, 1], f32)
            nc.any.tensor_copy(part[:], ps_s[:])

            # AllGather the 8 per-core partial sums (single collective op),
            # then reduce + broadcast locally.
            cin = dram.tile([1, 1], f32)
            cout = dram.tile([N_CORES, 1], f32, addr_space="Shared")
            nc.gpsimd.dma_start(cin[:], part[:])
            nc.gpsimd.collective_compute(
                "AllGather",
                mybir.AluOpType.bypass,
                ins=[cin.opt()],
                outs=[cout.opt()],
                replica_groups=[list(range(N_CORES))],
            )
            # broadcast the 8 partials to all 128 partitions and sum them:
            # threshold = sum * (1/count) * 0.7
            parts128 = const.tile([128, N_CORES], f32)
            nc.gpsimd.dma_start(
                parts128[:],
                cout[:].rearrange("a b -> b a").to_broadcast((128, N_CORES)),
            )
            tot128 = const.tile([128, 1], f32)
            nc.vector.reduce_sum(tot128[:], parts128[:], axis=mybir.AxisListType.X)
            thr = const.tile([128, 1], f32)
            nc.vector.tensor_scalar(
                thr[:],
                tot128[:],
                float(np.float32(1.0) / np.float32(W_COUNT)),
                THRESH_FACTOR,
                mybir.AluOpType.mult,
                mybir.AluOpType.mult,
            )
            nthr = const.tile([128, 1], f32)
            nc.vector.tensor_scalar_mul(nthr[:], thr[:], -1.0)

            # ---- phase Q: ternarize into resident bf16 Wq^T
            # wq = sign(w - clamp(w, -thr, thr)): exactly 0 for |w| <= thr,
            # else +-1. clamp+sub on DVE, sign on ScalarE (parallel engines).
            # The second W pass prefetches into its own pool so the DMAs run
            # during the collective wait.
            wq_sb = wqp.tile([128, KT, O_PAD], f16)
            for k in range(KT):
                w_k = wld.tile([128, O_PAD], f32, tag="wld")
                nc.sync.dma_start(w_k[:], wt_t[:, k])
                cl = clp.tile([128, O_PAD], f32, tag="cl")
                nc.vector.tensor_scalar(
                    cl[:],
                    w_k[:],
                    thr[:],
                    nthr[:],
                    mybir.AluOpType.min,
                    mybir.AluOpType.max,
                )
                df = qtmp.tile([128, O_PAD], bf16, tag="df")
                nc.vector.tensor_tensor(
                    df[:], w_k[:], cl[:], mybir.AluOpType.subtract
                )
                nc.scalar.sign(wq_sb[:, k, :], df[:])

            # ---- phase M: tiled matmul, x stationary / Wq moving
            for mo in range(MT):
                xt_sb = xin.tile([128, KT, 128], f32)
                nc.sync.dma_start(xt_sb[:], xt[mo])
                xb = xbp.tile([128, KT, 128], f16, tag="hi")
                nc.vector.tensor_copy(xb[:], xt_sb[:])
                if split_lo:
                    xl = xbp.tile([128, KT, 128], f16, tag="lo")
                    nc.vector.tensor_tensor(
                        xl[:], xt_sb[:], xb[:], mybir.AluOpType.subtract
                    )
                ps = [
                    mmps.tile([128, 512], f32, tag=f"p{ci}", name=f"ps{ci}")
                    for ci in range(len(O_CHUNKS))
                ]
                for k in range(KT):
                    for ci, (o0, w) in enumerate(O_CHUNKS):
                        nc.tensor.matmul(
                            ps[ci][:, :w],
                            lhsT=xb[:, k, :],
                            rhs=wq_sb[:, k, o0 : o0 + w],
                            start=(k == 0),
                            stop=(k == KT - 1 and not split_lo),
                        )
                        if split_lo:
                            nc.tensor.matmul(
                                ps[ci][:, :w],
                                lhsT=xl[:, k, :],
                                rhs=wq_sb[:, k, o0 : o0 + w],
                                start=False,
                                stop=(k == KT - 1),
                            )
                yr = yout.tile([128, O_PAD], f32)
                for ci, (o0, w) in enumerate(O_CHUNKS):
                    nc.vector.tensor_tensor(
                        yr[:, o0 : o0 + w],
                        ps[ci][:, :w],
                        scale_sb[:, o0 : o0 + w],
                        mybir.AluOpType.mult,
                    )
                nc.sync.dma_start(y[mo * 128 : (mo + 1) * 128, :], yr[:])

    nc.compile()
    return nc


def _get_nc(split_lo: bool):
    if split_lo not in _nc_cache:
        _nc_cache[split_lo] = _build(split_lo)
    return _nc_cache[split_lo]


def _prep_inputs(x: np.ndarray, weight: np.ndarray, scale: np.ndarray):
    xf = np.ascontiguousarray(x, dtype=np.float32).reshape(M, K)
    # xt[mo, ki, ko, mi] = x[mo*128+mi, ko*128+ki]
    xt = np.ascontiguousarray(xf.reshape(MT, 128, KT, 128).transpose(0, 3, 2, 1))
    in_maps = []
    for c in range(N_CORES):
        wsl = weight[c * O_SLICE : (c + 1) * O_SLICE].astype(np.float32, copy=False)
        wp = np.zeros((O_PAD, K), dtype=np.float32)
        wp[:O_SLICE] = wsl
        wt = np.ascontiguousarray(wp.T)  # [K, O_PAD]
        ssl = scale[c * O_SLICE : (c + 1) * O_SLICE].astype(np.float32, copy=False)
        sp = np.zeros((O_PAD,), dtype=np.float32)
        sp[:O_SLICE] = ssl.reshape(-1)
        sc = np.ascontiguousarray(np.broadcast_to(sp[None, :], (128, O_PAD)))
        in_maps.append({"xt": xt, "wt": wt, "sc": sc})
    return in_maps


def _run(x, weight, scale, split_lo=None, **run_kwargs):
    if split_lo is None:
        split_lo = SPLIT_LO
    nc = _get_nc(split_lo)
    in_maps = _prep_inputs(x, weight, scale)
    res = run_bass_kernel_spmd(nc, in_maps, core_ids=list(range(N_CORES)), **run_kwargs)
    parts = [res.results[c]["y"][:, :O_SLICE] for c in range(N_CORES)]
    y = np.concatenate(parts, axis=1).reshape(4, 2048, O_FULL).astype(np.float32)
    return y, res


def kernel(x: np.ndarray, weight: np.ndarray, scale: np.ndarray) -> np.ndarray:
    y, _ = _run(x, weight, scale)
    return y


# revision 24
# speedup vs baseline: 1.0307x; 1.0003x over previous
"""BitLinear forward on 8 Trainium2 NeuronCores.

Computation (reference):
    threshold = mean(|W|) * 0.7            (global scalar over full W)
    Wq = sign(W) * (|W| > threshold)       (ternary {-1, 0, 1})
    y = x @ (Wq * scale).T                 (x: [4, 2048, 4096], W: [11008, 4096])

Sharding: column-parallel over out_features. Each core owns a 1376-row slice
of W (zero-padded to 1408 = 11*128), gets the full x, and computes its slice
of the output. The global mean needs a cross-core AllReduce of one scalar.

On-device pipeline per core:
    T: stream W^T tiles, |.|-reduce to a partial sum, AllReduce -> threshold
    Q: re-stream W^T tiles, ternarize to a resident bf16 Wq^T in SBUF (exact)
    M: for each 128-row tile of x: cast x to bf16 (optionally hi+lo split),
       matmul (x tile stationary, Wq^T moving) accumulating over K in PSUM,
       apply scale on PSUM eviction, DMA out.

Matmul dtype: bf16. Wq is exactly representable in bf16 (ternary), scale is
applied in fp32 on the PSUM output, so the only quantization is the x cast.
With SPLIT_LO=True, x is split as x = hi + lo (two bf16 matmuls accumulating
in the same fp32 PSUM) for ~2^-18 relative x error at 2x PE cost.
"""

import numpy as np

import concourse.mybir as mybir
import concourse.tile as tile
from concourse import bacc
from concourse import bass_utils as _bass_utils
from concourse.bass_utils import run_bass_kernel_spmd
from concourse.tile import add_dep_helper

# note: walrus --enable-ldw-opt=true rejects bass-emitted standalone
# InstLdweights ("not compatible with LDW optimization"), so the per-matmul
# ~107ns weight load cannot be optimized away at the compiler level.
_ = _bass_utils

N_CORES = 8
O_FULL = 11008
K = 4096
M = 8192
O_SLICE = O_FULL // N_CORES  # 1376
O_PAD = 1408  # 11 * 128
KT = K // 128  # 32
MT = M // 128  # 64
O_CHUNKS = ((0, 512), (512, 512), (1024, 384))
W_COUNT = float(O_FULL) * float(K)
THRESH_FACTOR = 0.7

SPLIT_LO = False  # x = hi + lo f16 split (2x PE work, ~fp32 accuracy)
X_RAW = False  # x stationary as float32r (no cast; full x precision if HW allows)

_nc_cache = {}


def _build(split_lo: bool, x_raw: bool = False):
    nc = bacc.Bacc(None, target_bir_lowering=False)
    f32 = mybir.dt.float32
    bf16 = mybir.dt.bfloat16
    f16 = mybir.dt.float16
    f32r = mybir.dt.float32r

    # x pre-tiled on host: xt[mo, ki, ko, mi] = x[mo*128+mi, ko*128+ki]
    xt = nc.dram_tensor(
        "xt", [MT, 128, KT, 128], f32r if x_raw else f32, kind="ExternalInput"
    )
    # W slice transposed: wt[i, o] = W[o_global, i], zero-padded to O_PAD
    wt = nc.dram_tensor("wt", [K, O_PAD], f32, kind="ExternalInput")
    # scale slice replicated to 128 partitions on host
    sc = nc.dram_tensor("sc", [128, O_PAD], f32, kind="ExternalInput")
    y = nc.dram_tensor("y", [M, O_PAD], f32, kind="ExternalOutput")

    wt_t = wt[:].rearrange("(ko ki) o -> ki ko o", ki=128)  # [128, KT, O_PAD]

    with tile.TileContext(nc) as tc:
        with (
            tc.tile_pool(name="const", bufs=1) as const,
            tc.tile_pool(name="wld", bufs=10) as wld,
            tc.tile_pool(name="qtmp", bufs=2) as qtmp,
            tc.tile_pool(name="clp", bufs=1) as clp,
            tc.tile_pool(name="wq", bufs=1) as wqp,
            tc.tile_pool(name="xin", bufs=1) as xin,
            tc.tile_pool(name="xbp", bufs=2) as xbp,
            tc.tile_pool(name="yout", bufs=2) as yout,
            tc.tile_pool(name="mm_psum", bufs=2, space="PSUM") as mmps,
            tc.tile_pool(name="sc_psum", bufs=1, space="PSUM") as scps,
            tc.tile_pool(name="dram", bufs=1, space="DRAM") as dram,
        ):
            ones = const.tile([128, 1], f32)
            nc.any.memset(ones[:], 1.0)
            scale_sb = const.tile([128, O_PAD], f32)
            nc.sync.dma_start(scale_sb[:], sc[:])

            # ---- phase T: partial sum of |W| on this core
            acc = const.tile([128, KT], f32)
            last_t_dma = None
            for k in range(KT):
                w_k = wld.tile([128, O_PAD], f32, tag="wld")
                last_t_dma = nc.sync.dma_start(w_k[:], wt_t[:, k])
                nc.vector.reduce_sum(
                    acc[:, k : k + 1],
                    w_k[:],
                    axis=mybir.AxisListType.X,
                    apply_absolute_value=True,
                )
            red = const.tile([128, 1], f32)
            nc.vector.reduce_sum(red[:], acc[:], axis=mybir.AxisListType.X)
            ps_s = scps.tile([1, 1], f32, tag="s")
            nc.tensor.matmul(ps_s[:], lhsT=ones[:], rhs=red[:], start=True, stop=True)
            part = const.tile([1, 1], f32)
            nc.any.tensor_copy(part[:], ps_s[:])

            # AllGather the 8 per-core partial sums (single collective op),
            # then reduce + broadcast locally.
            cin = dram.tile([1, 1], f32)
            cout = dram.tile([N_CORES, 1], f32, addr_space="Shared")
            nc.gpsimd.dma_start(cin[:], part[:])
            nc.gpsimd.collective_compute(
                "AllGather",
                mybir.AluOpType.bypass,
                ins=[cin.opt()],
                outs=[cout.opt()],
                replica_groups=[list(range(N_CORES))],
            )
            # broadcast the 8 partials to all 128 partitions and sum them:
            # threshold = sum * (1/count) * 0.7
            parts128 = const.tile([128, N_CORES], f32)
            nc.gpsimd.dma_start(
                parts128[:],
                cout[:].rearrange("a b -> b a").to_broadcast((128, N_CORES)),
            )
            tot128 = const.tile([128, 1], f32)
            nc.vector.reduce_sum(tot128[:], parts128[:], axis=mybir.AxisListType.X)
            thr = const.tile([128, 1], f32)
            nc.vector.tensor_scalar(
                thr[:],
                tot128[:],
                float(np.float32(1.0) / np.float32(W_COUNT)),
                THRESH_FACTOR,
                mybir.AluOpType.mult,
                mybir.AluOpType.mult,
            )
            nthr = const.tile([128, 1], f32)
            nc.vector.tensor_scalar_mul(nthr[:], thr[:], -1.0)

            # ---- phase Q: ternarize into resident bf16 Wq^T
            # wq = sign(w - clamp(w, -thr, thr)): exactly 0 for |w| <= thr,
            # else +-1. clamp+sub on DVE, sign on ScalarE (parallel engines).
            # The second W pass prefetches into its own pool so the DMAs run
            # during the collective wait.
            wq_sb = wqp.tile([128, KT, O_PAD], f16)
            for k in range(KT):
                w_k = wld.tile([128, O_PAD], f32, tag="wld")
                q_dma = nc.sync.dma_start(w_k[:], wt_t[:, k])
                # keep the T pass (threshold critical path) at full HBM BW:
                # the re-read may only start once the first pass is issued
                add_dep_helper(
                    q_dma.ins, last_t_dma.ins, False, "W re-read after T pass"
                )
                cl = clp.tile([128, O_PAD], f32, tag="cl")
                nc.vector.tensor_scalar(
                    cl[:],
                    w_k[:],
                    thr[:],
                    nthr[:],
                    mybir.AluOpType.min,
                    mybir.AluOpType.max,
                )
                df = qtmp.tile([128, O_PAD], bf16, tag="df")
                nc.vector.tensor_tensor(
                    df[:], w_k[:], cl[:], mybir.AluOpType.subtract
                )
                nc.scalar.sign(wq_sb[:, k, :], df[:])

            # ---- phase M: tiled matmul, x stationary / Wq moving
            for mo in range(MT):
                xt_sb = xin.tile([128, KT, 128], f32r if x_raw else f32)
                nc.sync.dma_start(xt_sb[:], xt[mo])
                if x_raw:
                    xb = xt_sb
                else:
                    xb = xbp.tile([128, KT, 128], f16, tag="hi")
                    nc.vector.tensor_copy(xb[:], xt_sb[:])
                if split_lo:
                    xl = xbp.tile([128, KT, 128], f16, tag="lo")
                    nc.vector.tensor_tensor(
                        xl[:], xt_sb[:], xb[:], mybir.AluOpType.subtract
                    )
                ps = [
                    mmps.tile([128, 512], f32, tag=f"p{ci}", name=f"ps{ci}")
                    for ci in range(len(O_CHUNKS))
                ]
                for k in range(KT):
                    for ci, (o0, w) in enumerate(O_CHUNKS):
                        nc.tensor.matmul(
                            ps[ci][:, :w],
                            lhsT=xb[:, k, :],
                            rhs=wq_sb[:, k, o0 : o0 + w],
                            start=(k == 0),
                            stop=(k == KT - 1 and not split_lo),
                        )
                        if split_lo:
                            nc.tensor.matmul(
                                ps[ci][:, :w],
                                lhsT=xl[:, k, :],
                                rhs=wq_sb[:, k, o0 : o0 + w],
                                start=False,
                                stop=(k == KT - 1),
                            )
                yr = yout.tile([128, O_PAD], f32)
                for ci, (o0, w) in enumerate(O_CHUNKS):
                    nc.vector.tensor_tensor(
                        yr[:, o0 : o0 + w],
                        ps[ci][:, :w],
                        scale_sb[:, o0 : o0 + w],
                        mybir.AluOpType.mult,
                    )
                nc.sync.dma_start(y[mo * 128 : (mo + 1) * 128, :], yr[:])

    nc.compile()
    return nc


def _get_nc(split_lo: bool, x_raw: bool = False):
    key = (split_lo, x_raw)
    if key not in _nc_cache:
        _nc_cache[key] = _build(split_lo, x_raw)
    return _nc_cache[key]


def _prep_inputs(x: np.ndarray, weight: np.ndarray, scale: np.ndarray):
    xf = np.ascontiguousarray(x, dtype=np.float32).reshape(M, K)
    # xt[mo, ki, ko, mi] = x[mo*128+mi, ko*128+ki]
    xt = np.ascontiguousarray(xf.reshape(MT, 128, KT, 128).transpose(0, 3, 2, 1))
    in_maps = []
    for c in range(N_CORES):
        wsl = weight[c * O_SLICE : (c + 1) * O_SLICE].astype(np.float32, copy=False)
        wp = np.zeros((O_PAD, K), dtype=np.float32)
        wp[:O_SLICE] = wsl
        wt = np.ascontiguousarray(wp.T)  # [K, O_PAD]
        ssl = scale[c * O_SLICE : (c + 1) * O_SLICE].astype(np.float32, copy=False)
        sp = np.zeros((O_PAD,), dtype=np.float32)
        sp[:O_SLICE] = ssl.reshape(-1)
        sc = np.ascontiguousarray(np.broadcast_to(sp[None, :], (128, O_PAD)))
        in_maps.append({"xt": xt, "wt": wt, "sc": sc})
    return in_maps


def _run(x, weight, scale, split_lo=None, x_raw=None, **run_kwargs):
    if split_lo is None:
        split_lo = SPLIT_LO
    if x_raw is None:
        x_raw = X_RAW
    nc = _get_nc(split_lo, x_raw)
    in_maps = _prep_inputs(x, weight, scale)
    res = run_bass_kernel_spmd(nc, in_maps, core_ids=list(range(N_CORES)), **run_kwargs)
    parts = [res.results[c]["y"][:, :O_SLICE] for c in range(N_CORES)]
    y = np.concatenate(parts, axis=1).reshape(4, 2048, O_FULL).astype(np.float32)
    return y, res


def kernel(x: np.ndarray, weight: np.ndarray, scale: np.ndarray) -> np.ndarray:
    y, _ = _run(x, weight, scale)
    return y
